# revision 14
# baseline (speedup 1.0000x reference)
"""Trainium2 Bass kernel for nn_Block_56616258896419 (moe_routing).

Self-contained: takes FULL inputs (as from setup_inputs()), returns FULL
[4,1024,1024] f32 output. Internally shards across 8 NeuronCores:
  - tokens 8-way (core r: batch r//2, sequence half r%2) for attention/LN
  - experts 8-way (core r computes expert r over ALL tokens) for the MoE
Heavy matmuls (QKV, proj, both expert GEMMs) run in fp8-e4m3 with
DoubleRow perf mode (2 MACs/cell/cycle). Scales fold into weight staging
(wqkv/wp/wr x64, w1 x16, w2 x32) and are undone inside existing ops:
LN1 is scale-invariant (residual x staged x64), the expert combine
multiplies by probs/512, q is scaled by 0.125/64^2 before softmax.
Collectives: 8-way AllGather of LN1'd activations (transposed, fp8),
4x 8-way ReduceScatter (bf16) of the prob-weighted expert outputs,
overlapped with MoE compute.
"""
import numpy as np
import ml_dtypes

B, S, E, H, HD, NEXP, FF = 4, 1024, 1024, 16, 64, 8, 4096
NCORE = 8
EPS = 1e-5
BF16 = ml_dtypes.bfloat16
FP8 = ml_dtypes.float8_e4m3

SW_QKV = 64.0
SW_P = 64.0
SW_R = 64.0
SW_1 = 16.0
SW_2 = 32.0
PCOL_DESCALE = 1.0 / (SW_1 * SW_2)
Q_SCALE = 0.125 / (SW_QKV * SW_QKV)
V_DESCALE = 1.0 / SW_QKV
R_DESCALE = 1.0 / SW_R

_CACHE = {}


def _build_program():
    import concourse.bacc as bacc
    import concourse.mybir as mybir
    import concourse.tile as tile

    dt = mybir.dt
    f32, bf, f8 = dt.float32, dt.bfloat16, dt.float8e4
    AF = mybir.ActivationFunctionType
    ALU = mybir.AluOpType
    DR = mybir.MatmulPerfMode.DoubleRow

    nc = bacc.Bacc("TRN2", target_bir_lowering=False, debug=False,
                   num_devices=NCORE)

    # ---------------- I/O ----------------
    def inp(name, shape, d):
        return nc.dram_tensor(name, shape, d, kind="ExternalInput").ap()

    xT_d = inp("xT", [128, 2 * 4096], f8)          # x^T [own|partner], e-tiled
    xr_d = inp("xr", [128, 4 * 1024], bf)         # 64*x token-major, tt blocks
    xr2_d = inp("xr2", [128, 4 * 1024], f32)       # x rows [b, 128r:128r+128]
    wqkv_d = inp("wqkv", [128, 8 * 3072], f8)      # 64*[E,3E] e-tiled
    bqk_d = inp("bqk", [128, 16], f32)             # 64*biases
    bv_d = inp("bv", [1, 1024], bf)                # 64*bv
    wp_d = inp("wp", [128, 8 * 1024], f8)          # 64*wp e-tiled
    bp_d = inp("bp", [1, 1024], bf)                # 64*bp
    masks_d = inp("masks", [4, 128, 256], bf)      # diag-pair 0/1 masks
    ident_d = inp("ident", [128, 128], bf)
    w1_d = inp("w1", [128, 8 * 4096], f8)          # 16*(ln1-folded), e-tiled
    b1_d = inp("b1", [128, 32], f32)               # 16*b1
    w2_d = inp("w2", [128, 32 * 1024], f8)         # 32*w2, ff-tiled
    b2_d = inp("b2", [1, 1024], bf)                # 512*b2
    wr_d = inp("wr", [128, 8 * 8], f8)             # 64*(ln1-folded), permuted
    br_d = inp("br", [1, 8], bf)                   # 64*br
    g2_d = inp("g2", [128, 1024], f32)             # ln2_g replicated
    bl2_d = inp("bl2", [128, 1024], f32)           # ln2_b replicated
    out_d = nc.dram_tensor("out", [4, 128, 1024], f32,
                           kind="ExternalOutput").ap()
    DBG = False
    if DBG:
        dbg_q = nc.dram_tensor("dbg_q", [128, 4096], bf,
                               kind="ExternalOutput").ap()
        dbg_k = nc.dram_tensor("dbg_k", [128, 8192], bf,
                               kind="ExternalOutput").ap()
        dbg_c = nc.dram_tensor("dbg_c", [128, 4096], f8,
                               kind="ExternalOutput").ap()
        dbg_h = nc.dram_tensor("dbg_h", [128, 4096], bf,
                               kind="ExternalOutput").ap()

    # ---------------- internal DRAM ----------------
    hag_inA = nc.dram_tensor("hag_inA", [8, 128, 256], f8).ap()
    hag_inB = nc.dram_tensor("hag_inB", [8, 128, 256], f8).ap()
    hag_outA = nc.dram_tensor("hag_outA", [8, 8, 128, 256], f8,
                              addr_space="Shared").ap()
    hag_outB = nc.dram_tensor("hag_outB", [8, 8, 128, 256], f8,
                              addr_space="Shared").ap()
    rs_ins = [nc.dram_tensor(f"rs_in{g}", [1024, 1024], bf).ap()
              for g in range(4)]
    rs_outs = [nc.dram_tensor(f"rs_out{g}", [128, 1024], bf).ap()
               for g in range(4)]

    with tile.TileContext(nc) as tc:
        cpool_cm = tc.tile_pool(name="cpool", bufs=1, side="left")
        cpool = cpool_cm.__enter__()
        ones_row = cpool.tile([1, 128], bf)
        nc.vector.memset(ones_row[:], 1.0)
        bqk_sb = cpool.tile([128, 16], f32)
        nc.sync.dma_start(bqk_sb[:], bqk_d[:])
        bv_sb = cpool.tile([1, 1024], bf)
        nc.sync.dma_start(bv_sb[:], bv_d[:])
        bp_sb = cpool.tile([1, 1024], bf)
        nc.sync.dma_start(bp_sb[:], bp_d[:])
        ident_sb = cpool.tile([128, 128], bf)
        nc.sync.dma_start(ident_sb[:], ident_d[:])
        wr_sb = cpool.tile([128, 64], f8)
        nc.sync.dma_start(wr_sb[:], wr_d[:])
        br_sb = cpool.tile([1, 8], bf)
        nc.sync.dma_start(br_sb[:], br_d[:])
        b1_sb = cpool.tile([128, 32], f32)
        nc.sync.dma_start(b1_sb[:], b1_d[:])
        b2_sb = cpool.tile([1, 1024], bf)
        nc.sync.dma_start(b2_sb[:], b2_d[:])

        # ===== phase 1+2: fused QKV + attention (q-block ss outer) =====
        qkv_cm = tc.tile_pool(name="qkv", bufs=1, side="right")
        qkv = qkv_cm.__enter__()
        xT_sb = qkv.tile([128, 8192], f8)
        nc.sync.dma_start(xT_sb[:], xT_d[:])
        wqkv_sb = qkv.tile([128, 24576], f8)
        for sect in (1, 2, 0):            # k first, then v, then q
            nc.sync.dma_start(
                wqkv_sb[:].rearrange("p (e s c) -> p e s c", e=8, s=3)[:, :, sect],
                wqkv_d[:].rearrange("p (e s c) -> p e s c", e=8, s=3)[:, :, sect])

        attn_cm = tc.tile_pool(name="attn", bufs=1, side="right")
        attn = attn_cm.__enter__()
        qT_sb = attn.tile([128, 4096], bf)
        kT_full = attn.tile([128, 8192], bf)     # [j][half*512 + s], 64-scaled
        # augmented V: per t-tile, 16 heads x (64 v-cols + 1 ones-col)
        v_aug = attn.tile([128, 8 * 1040], bf)
        for u in range(8):
            nc.vector.memset(
                v_aug[:, u * 1040: u * 1040 + 1040]
                .rearrange("p (h d) -> p h d", d=65)[:, :, 64:65], 1.0)
        mask_sb = attn.tile([128, 1024], bf)
        for ss in range(4):
            nc.sync.dma_start(mask_sb[:, ss * 256: ss * 256 + 256],
                              masks_d[ss])

        proj_cm = tc.tile_pool(name="proj", bufs=1, side="right")
        projp = proj_cm.__enter__()
        cat8_sb = projp.tile([128, 4096], f8)    # normalized heads^T (fp8)
        wp_sb = projp.tile([128, 8192], f8)
        nc.sync.dma_start(wp_sb[:], wp_d[:])
        x_sb = projp.tile([128, 4096], bf)       # 64*x
        nc.sync.dma_start(x_sb[:], xr_d[:])
        h_sb = projp.tile([128, 4096], bf)
        hT_stage = projp.tile([128, 4096], f8)

        # MoE weights prefetch - overlaps the whole attention phase
        moe_cm = tc.tile_pool(name="moe", bufs=1, side="left")
        moe = moe_cm.__enter__()
        w1_sb = moe.tile([128, 32768], f8)
        w2_sb = moe.tile([128, 32768], f8)
        for et in range(8):
            nc.sync.dma_start(w1_sb[:, et * 4096: et * 4096 + 4096],
                              w1_d[:, et * 4096: et * 4096 + 4096])
        for ft8 in range(8):
            nc.sync.dma_start(w2_sb[:, ft8 * 4096: ft8 * 4096 + 4096],
                              w2_d[:, ft8 * 4096: ft8 * 4096 + 4096])

        hTcp_cm = tc.tile_pool(name="hTcp", bufs=2, side="left")
        hTcp = hTcp_cm.__enter__()

        xT4 = xT_sb[:].rearrange("p (hf et t) -> p hf et t", hf=2, et=8)
        wq4 = wqkv_sb[:].rearrange("p (et s c) -> p et s c", et=8, s=3)
        cat4 = cat8_sb[:].rearrange("p (s2 jj t) -> p s2 jj t", s2=4, jj=8)
        wp4 = wp_sb[:].rearrange("p (et c) -> p et c", et=8)
        hT3 = hT_stage[:].rearrange("p (et t) -> p et t", et=8)

        # block order within a subtile: non-diag (ph*ss+tl), diag at 2ss+ph
        def blk(ss, ph, tl):
            return 2 * ss + ph if tl == ss else ph * ss + tl

        # v c1 emission order inside ss=0 (u needed by av(ss,.): u in
        # {tl, 4+tl : tl<=ss}; c1 heads 8..15 needed from j=4 of each ss)
        V1_ORDER = [0, 4, 1, 5, 2, 6, 3, 7]

        def emit_v(half, tt, c, scp2, ps_sc2):
            v_ps = ps_sc2.tile([128, 1024], f32, tag="sc",
                               name=f"v{half}_{tt}_{c}")[:, 0:512]
            for g in range(4):
                nc.tensor.matmul(
                    v_ps[:],
                    xT4[:, half, 2 * g:2 * g + 2, tt * 128: tt * 128 + 128],
                    wq4[:, 2 * g:2 * g + 2, 2, c * 512: c * 512 + 512],
                    start=(g == 0), stop=False, perf_mode=DR)
            nc.tensor.matmul(
                v_ps[:], ones_row[:, 0:128],
                bv_sb[:, c * 512: c * 512 + 512],
                start=False, stop=True)
            u = half * 4 + tt
            v_st = scp2.tile([128, 512], bf, tag="v_st", bufs=2,
                             name=f"vs{half}_{tt}_{c}")
            nc.vector.tensor_scalar_mul(v_st[:], v_ps[:], V_DESCALE)
            nc.vector.tensor_copy(
                v_aug[:, u * 1040: u * 1040 + 1040]
                .rearrange("p (h d) -> p h d", d=65)[:, 8 * c: 8 * c + 8, 0:64],
                v_st[:].rearrange("p (h d) -> p h d", d=64))

        with tc.tile_pool(name="sc", bufs=4, side="left") as scp, \
             tc.tile_pool(name="ps_sc", bufs=2, space="PSUM") as ps_sc, \
             tc.tile_pool(name="ps_av", bufs=2, space="PSUM") as ps_av, \
             tc.tile_pool(name="ps_pr", bufs=1, space="PSUM") as ps_pr:
            # v c0 for all 8 u-tiles up front (needed by av(0, j<4))
            for half in range(2):
                for tt in range(4):
                    emit_v(half, tt, 0, scp, ps_sc)

            av_tiles = {}
            exp_tiles = {}
            preloaded = {}

            def preload_chunk(ci, q, hf):
                t = hTcp.tile([128, 4096], f8, tag="hT_c", name=f"hTc{ci}")
                hsrc = hag_outA if hf == 0 else hag_outB
                for et in range(8):
                    for si in range(2):
                        nc.gpsimd.dma_start(
                            t[:, et * 512 + si * 256:
                              et * 512 + si * 256 + 256],
                            hsrc[2 * q + si, et])
                preloaded[ci] = t

            def emit_sc_exp(ss, j):
                scs = []
                for h01 in range(2):
                    scs.append(ps_sc.tile([128, 1024], f32, tag="sc",
                                          name=f"sc{ss}_{j}_{h01}"))
                for ph in range(2):
                    for tl in range(ss + 1):
                        m = blk(ss, ph, tl)
                        for h01 in range(2):
                            po = 64 * h01
                            nc.tensor.matmul(
                                scs[h01][:, m * 128: m * 128 + 128],
                                kT_full[po:po + 64,
                                        j * 1024 + ph * 512 + tl * 128:
                                        j * 1024 + ph * 512 + tl * 128 + 128],
                                qT_sb[po:po + 64, j * 512 + ss * 128:
                                      j * 512 + ss * 128 + 128],
                                start=True, stop=True)
                exps = []
                for h01 in range(2):
                    expT = scp.tile([128, 1024], bf, tag="expT", bufs=3,
                                    name=f"ex{ss}_{j}_{h01}")
                    exps.append(expT)
                    nc.scalar.activation(
                        expT[:, 0: (2 * ss + 2) * 128],
                        scs[h01][:, 0: (2 * ss + 2) * 128], AF.Exp)
                    nc.vector.tensor_tensor(
                        expT[:, 2 * ss * 128: 2 * ss * 128 + 256],
                        expT[:, 2 * ss * 128: 2 * ss * 128 + 256],
                        mask_sb[:, ss * 256: ss * 256 + 256], op=ALU.mult)
                exp_tiles[(j % 4, 0)] = exps[0]
                exp_tiles[(j % 4, 1)] = exps[1]

            def emit_av(ss, j):
                if j % 4 == 0:
                    av_tiles[0] = ps_av.tile([65, 512], f32, tag="av",
                                             name=f"av{ss}_{j // 4}_0")
                    av_tiles[1] = ps_av.tile([65, 512], f32, tag="av",
                                             name=f"av{ss}_{j // 4}_1")
                exps = [exp_tiles[(j % 4, 0)], exp_tiles[(j % 4, 1)]]
                nblk = 2 * (ss + 1)
                bi = 0
                for ph in range(2):
                    for tl in range(ss + 1):
                        m = blk(ss, ph, tl)
                        u = ph * 4 + tl
                        bi += 1
                        for h01 in range(2):
                            h = 2 * j + h01
                            nc.tensor.matmul(
                                av_tiles[h01][:, (j % 4) * 128:
                                              (j % 4) * 128 + 128],
                                v_aug[:, u * 1040 + h * 65:
                                      u * 1040 + h * 65 + 65],
                                exps[h01][:, m * 128: m * 128 + 128],
                                start=(bi == 1), stop=(bi == nblk))

            def emit_cat(ss, jh):
                # normalize 4 head-pairs' q-slice from av PSUM into cat8.
                # cat8 layout is ss-major: col = ss*1024 + j*128 + t, so this
                # drain is a contiguous [128, 512] block (2D ops only; ACT
                # stages the PSUM reads into SBUF as in v2).
                bc = ps_sc.tile([128, 1024], f32, tag="sc",
                                name=f"bc{ss}_{jh}")[:, 0:512]
                catT_st = scp.tile([128, 512], bf, tag="catT", bufs=2,
                                   name=f"cst{ss}_{jh}")
                for h01 in range(2):
                    nc.scalar.copy(catT_st[64 * h01: 64 * h01 + 64, :],
                                   av_tiles[h01][0:64, :])
                    sm_f = scp.tile([1, 512], f32, tag="sm_f", bufs=1,
                                    name=f"smf{ss}_{jh}_{h01}")
                    nc.scalar.copy(sm_f[:], av_tiles[h01][64:65, :])
                    rc_f = scp.tile([1, 512], f32, tag="rc_f", bufs=1,
                                    name=f"rcf{ss}_{jh}_{h01}")
                    nc.vector.reciprocal_approx_fast(rc_f[:], sm_f[:])
                    rc_h = scp.tile([1, 512], bf, tag="rc_h", bufs=1,
                                    name=f"rc{ss}_{jh}_{h01}")
                    nc.vector.tensor_copy(rc_h[:], rc_f[:])
                    nc.tensor.matmul(bc[64 * h01: 64 * h01 + 64, :],
                                     ones_row[:, 0:64], rc_h[:],
                                     start=True, stop=True)
                nc.vector.tensor_tensor(
                    cat8_sb[:, ss * 1024 + jh * 512:
                            ss * 1024 + jh * 512 + 512],
                    catT_st[:], bc[:], op=ALU.mult)

            def emit_proj_ln(ss):
                tt = ss
                y_sb = prp.tile([128, 1024], f32, tag="y", bufs=1)
                for ec in range(2):
                    ao_ps = ps_pr.tile([128, 512], f32, tag="ao")
                    for g in range(4):
                        nc.tensor.matmul(
                            ao_ps[:],
                            cat4[:, tt, 2 * g:2 * g + 2, :],
                            wp4[:, 2 * g:2 * g + 2, ec * 512: ec * 512 + 512],
                            start=(g == 0), stop=False, perf_mode=DR)
                    nc.tensor.matmul(
                        ao_ps[:], ones_row[:, 0:128],
                        bp_sb[:, ec * 512: ec * 512 + 512],
                        start=False, stop=True)
                    nc.vector.tensor_tensor(
                        y_sb[:, ec * 512: ec * 512 + 512], ao_ps[:],
                        x_sb[:, tt * 1024 + ec * 512: tt * 1024 + ec * 512 + 512],
                        op=ALU.add)
                # LN1 stats (y is 64*(attn_out+x); LN is scale-invariant)
                mean = prp.tile([128, 1], f32, tag="mean")
                nc.vector.reduce_sum(mean[:], y_sb[:], axis=mybir.AxisListType.X)
                nc.vector.tensor_scalar_mul(mean[:], mean[:], 1.0 / 1024.0)
                sq = scp.tile([128, 1024], bf, tag="expT", bufs=3,
                              name=f"sq{ss}")
                sqs = prp.tile([128, 1], f32, tag="sqs")
                nc.scalar.activation(sq[:], y_sb[:], AF.Square,
                                     accum_out=sqs[:])
                m2 = prp.tile([128, 1], f32, tag="m2")
                nc.scalar.activation(m2[:], mean[:], AF.Square)
                var = prp.tile([128, 1], f32, tag="var")
                nc.vector.tensor_scalar(var[:], sqs[:], 1.0 / 1024.0, EPS,
                                        op0=ALU.mult, op1=ALU.add)
                nc.vector.tensor_tensor(var[:], var[:], m2[:], op=ALU.subtract)
                std = prp.tile([128, 1], f32, tag="std")
                nc.scalar.activation(std[:], var[:], AF.Sqrt)
                rstd = prp.tile([128, 1], f32, tag="rstd")
                nc.vector.reciprocal(rstd[:], std[:])
                nc.vector.tensor_scalar(
                    h_sb[:, tt * 1024: tt * 1024 + 1024], y_sb[:],
                    mean[:], rstd[:], op0=ALU.subtract, op1=ALU.mult)
                # transpose h tile -> hT (fp8 via the copy)
                tp = ps_pr.tile([128, 1024], bf, tag="tp")
                for et in range(8):
                    nc.tensor.transpose(
                        tp[:, et * 128: et * 128 + 128],
                        h_sb[:, tt * 1024 + et * 128: tt * 1024 + et * 128 + 128],
                        ident_sb[:])
                nc.scalar.copy(
                    hT3[:, :, tt * 128: tt * 128 + 128],
                    tp[:].rearrange("p (et t) -> p et t", et=8))
                if tt == 1:
                    for et in range(8):
                        nc.gpsimd.dma_start(
                            hag_inA[et],
                            hT_stage[:, et * 512: et * 512 + 256])
                    nc.gpsimd.collective_compute(
                        "AllGather", mybir.AluOpType.bypass,
                        replica_groups=[list(range(8))],
                        ins=[hag_inA.opt()], outs=[hag_outA.opt()])
                    preload_chunk(0, 0, 0)
                    preload_chunk(1, 1, 0)
                if tt == 3:
                    for et in range(8):
                        nc.gpsimd.dma_start(
                            hag_inB[et],
                            hT_stage[:, et * 512 + 256: et * 512 + 512])
                    nc.gpsimd.collective_compute(
                        "AllGather", mybir.AluOpType.bypass,
                        replica_groups=[list(range(8))],
                        ins=[hag_inB.opt()], outs=[hag_outB.opt()])

            prp_cm = tc.tile_pool(name="prw", bufs=2, side="left")
            prp = prp_cm.__enter__()
            for ss in range(4):
                for j in range(8):
                    if ss == 0:
                        # interleave K (both halves) / Q / V-c1 for this j
                        k_dsts = [
                            (1, kT_full[:, j * 1024: j * 1024 + 512], 8 + j),
                            (1, kT_full[:, j * 1024 + 512: j * 1024 + 1024],
                             8 + j),
                            (0, qT_sb[:, j * 512: j * 512 + 512], j)]
                        for li, (sect, dst, bcol) in enumerate(k_dsts):
                            half = li if li < 2 else 0
                            kq_ps = ps_sc.tile([128, 1024], f32, tag="sc",
                                               name=f"kq{j}_{li}")[:, 0:512]
                            for g in range(4):
                                nc.tensor.matmul(
                                    kq_ps[:],
                                    wq4[:, 2 * g:2 * g + 2, sect,
                                        j * 128: j * 128 + 128],
                                    xT4[:, half, 2 * g:2 * g + 2, :],
                                    start=(g == 0), stop=(g == 3),
                                    perf_mode=DR)
                            if sect == 1:
                                nc.vector.tensor_scalar(
                                    dst, kq_ps[:],
                                    bqk_sb[:, bcol: bcol + 1], None,
                                    op0=ALU.add)
                            else:
                                nc.vector.tensor_scalar(
                                    dst, kq_ps[:],
                                    bqk_sb[:, bcol: bcol + 1], Q_SCALE,
                                    op0=ALU.add, op1=ALU.mult)
                        u = V1_ORDER[j]
                        emit_v(u // 4, u % 4, 1, scp, ps_sc)
                    emit_sc_exp(ss, j)
                    if ss >= 1:
                        # PE warm fillers: big DR matmuls keep HAM at speed
                        # through the ACT-bound softmax stretches
                        fl = ps_sc.tile([128, 1024], f32, tag="sc",
                                        name=f"fl{ss}_{j}")[:, 0:512]
                        for g in range(2):
                            nc.tensor.matmul(
                                fl[:],
                                wq4[:, 2 * g:2 * g + 2, 0, 0:128],
                                xT4[:, 0, 2 * g:2 * g + 2, :],
                                start=(g == 0), stop=(g == 1), perf_mode=DR)
                    if j >= 1:
                        emit_av(ss, j - 1)
                    if j == 3 + 1:
                        emit_cat(ss, 0)
                emit_av(ss, 7)
                emit_cat(ss, 1)
                emit_proj_ln(ss)
            prp_cm.__exit__(None, None, None)
            if DBG:
                nc.sync.dma_start(dbg_q[:], qT_sb[:])
                nc.sync.dma_start(dbg_k[:], kT_full[:])
                nc.sync.dma_start(dbg_c[:], cat8_sb[:])
                nc.sync.dma_start(dbg_h[:], h_sb[:])  # noqa
        proj_cm.__exit__(None, None, None)
        attn_cm.__exit__(None, None, None)
        qkv_cm.__exit__(None, None, None)

        # ===== phase 3: MoE (expert r over all tokens, 512-token chunks) ====
        # chunk (q, hf) = tokens of cores 2q,2q+1, local-token half hf
        w1v = w1_sb[:].rearrange("p (et f) -> p et f", et=8)
        w2v = w2_sb[:].rearrange("p (ft e) -> p ft e", ft=32)
        with tc.tile_pool(name="mchunk", bufs=2, side="left") as mck, \
             tc.tile_pool(name="ps_md", bufs=2, space="PSUM") as ps_md, \
             tc.tile_pool(name="ps_eo", bufs=2, space="PSUM") as ps_eo:
            chunks = [(0, 0), (1, 0), (0, 1), (1, 1),
                      (2, 0), (3, 0), (2, 1), (3, 1)]
            rs_fire = {2: 0, 3: 1, 6: 2, 7: 3}
            for ci, (q, hf) in enumerate(chunks):
                if ci in preloaded:
                    hT_c = preloaded.pop(ci)
                else:
                    hT_c = hTcp.tile([128, 4096], f8, tag="hT_c",
                                     name=f"hTc{ci}")
                    hsrc = hag_outA if hf == 0 else hag_outB
                    for et in range(8):
                        for si in range(2):
                            nc.gpsimd.dma_start(
                                hT_c[:, et * 512 + si * 256:
                                     et * 512 + si * 256 + 256],
                                hsrc[2 * q + si, et])
                h8v = hT_c[:].rearrange("p (et t) -> p et t", et=8)
                pcol = mck.tile([128, 4], f32, tag="pcol")
                for th in range(4):
                    lg_ps = ps_eo.tile([128, 8], f32, tag="lg")
                    for et in range(8):
                        nc.tensor.matmul(
                            lg_ps[:],
                            hT_c[:, et * 512 + th * 128: et * 512 + th * 128 + 128],
                            wr_sb[:, et * 8: et * 8 + 8],
                            start=(et == 0), stop=False)
                    nc.tensor.matmul(lg_ps[:], ones_row[:, 0:128], br_sb[:],
                                     start=False, stop=True)
                    pe = mck.tile([128, 8], f32, tag="pe")
                    ps = mck.tile([128, 1], f32, tag="ps")
                    nc.scalar.activation(pe[:], lg_ps[:], AF.Exp,
                                         scale=R_DESCALE, accum_out=ps[:])
                    pr = mck.tile([128, 1], f32, tag="pr")
                    nc.vector.reciprocal(pr[:], ps[:])
                    nc.vector.tensor_scalar(pcol[:, th:th + 1], pe[:, 0:1],
                                            pr[:], PCOL_DESCALE,
                                            op0=ALU.mult, op1=ALU.mult)
                midT8 = mck.tile([128, 16384], f8, tag="midT", bufs=2)
                for ft in range(32):
                    md_ps = ps_md.tile([128, 512], f32, tag="md")
                    for g in range(4):
                        nc.tensor.matmul(
                            md_ps[:],
                            w1v[:, 2 * g:2 * g + 2, ft * 128: ft * 128 + 128],
                            h8v[:, 2 * g:2 * g + 2, :],
                            start=(g == 0), stop=(g == 3), perf_mode=DR)
                    if ft % 2 == 0:
                        nc.scalar.activation(
                            midT8[:, ft * 512: ft * 512 + 512], md_ps[:],
                            AF.Relu, bias=b1_sb[:, ft: ft + 1])
                    else:
                        nc.vector.tensor_scalar(
                            midT8[:, ft * 512: ft * 512 + 512], md_ps[:],
                            b1_sb[:, ft: ft + 1], 0.0,
                            op0=ALU.add, op1=ALU.max)
                m8v = midT8[:].rearrange("p (ft t) -> p ft t", ft=32)
                eo_sb = mck.tile([128, 4096], bf, tag="eo", bufs=2)
                for th in range(4):
                    for ec in range(2):
                        eo_ps = ps_eo.tile([128, 512], f32, tag="eo_ps")
                        for g in range(16):
                            nc.tensor.matmul(
                                eo_ps[:],
                                m8v[:, 2 * g:2 * g + 2, th * 128: th * 128 + 128],
                                w2v[:, 2 * g:2 * g + 2, ec * 512: ec * 512 + 512],
                                start=(g == 0), stop=False, perf_mode=DR)
                        nc.tensor.matmul(
                            eo_ps[:], ones_row[:, 0:128],
                            b2_sb[:, ec * 512: ec * 512 + 512],
                            start=False, stop=True)
                        if ec == 0:
                            nc.scalar.activation(
                                eo_sb[:, th * 1024 + ec * 512:
                                      th * 1024 + ec * 512 + 512],
                                eo_ps[:], AF.Identity,
                                scale=pcol[:, th: th + 1])
                        else:
                            nc.vector.tensor_scalar_mul(
                                eo_sb[:, th * 1024 + ec * 512:
                                      th * 1024 + ec * 512 + 512],
                                eo_ps[:], pcol[:, th: th + 1])
                for th in range(4):
                    r0 = 512 * (th // 2) + hf * 256 + 128 * (th % 2)
                    nc.sync.dma_start(
                        rs_ins[q][r0: r0 + 128, :],
                        eo_sb[:, th * 1024: th * 1024 + 1024])
                if ci in rs_fire:
                    g = rs_fire[ci]
                    nc.gpsimd.collective_compute(
                        "ReduceScatter", mybir.AluOpType.add,
                        replica_groups=[list(range(8))],
                        ins=[rs_ins[g].opt()], outs=[rs_outs[g].opt()])
        hTcp_cm.__exit__(None, None, None)
        moe_cm.__exit__(None, None, None)

        # ============ phase 4: residual + LN2 (per RS group/batch) ============
        with tc.tile_pool(name="fin", bufs=2, side="left") as fin:
            x2_sb = fin.tile([128, 4096], f32, bufs=1)
            nc.sync.dma_start(x2_sb[:], xr2_d[:])
            g2_sb = fin.tile([128, 1024], f32, bufs=1)
            nc.sync.dma_start(g2_sb[:], g2_d[:])
            bl2_sb = fin.tile([128, 1024], f32, bufs=1)
            nc.sync.dma_start(bl2_sb[:], bl2_d[:])
            for g in range(4):
                y2b = fin.tile([128, 1024], bf, tag="y2b")
                nc.sync.dma_start(y2b[:], rs_outs[g][:])
                y2 = fin.tile([128, 1024], f32, tag="y2")
                nc.vector.tensor_tensor(
                    y2[:], y2b[:], x2_sb[:, g * 1024: g * 1024 + 1024],
                    op=ALU.add)
                mean = fin.tile([128, 1], f32, tag="mean2")
                nc.vector.reduce_sum(mean[:], y2[:], axis=mybir.AxisListType.X)
                nc.vector.tensor_scalar_mul(mean[:], mean[:], 1.0 / 1024.0)
                sq = fin.tile([128, 1024], f32, tag="sq2")
                sqs = fin.tile([128, 1], f32, tag="sqs2")
                nc.scalar.activation(sq[:], y2[:], AF.Square, accum_out=sqs[:])
                m2 = fin.tile([128, 1], f32, tag="m22")
                nc.scalar.activation(m2[:], mean[:], AF.Square)
                var = fin.tile([128, 1], f32, tag="var2")
                nc.vector.tensor_scalar(var[:], sqs[:], 1.0 / 1024.0, EPS,
                                        op0=ALU.mult, op1=ALU.add)
                nc.vector.tensor_tensor(var[:], var[:], m2[:], op=ALU.subtract)
                std = fin.tile([128, 1], f32, tag="std2")
                nc.scalar.activation(std[:], var[:], AF.Sqrt)
                rstd = fin.tile([128, 1], f32, tag="rstd2")
                nc.vector.reciprocal(rstd[:], std[:])
                on = fin.tile([128, 1024], f32, tag="on")
                nc.vector.tensor_scalar(on[:], y2[:], mean[:], rstd[:],
                                        op0=ALU.subtract, op1=ALU.mult)
                nc.vector.tensor_tensor(on[:], on[:], g2_sb[:], op=ALU.mult)
                nc.vector.tensor_tensor(on[:], on[:], bl2_sb[:], op=ALU.add)
                nc.sync.dma_start(out_d[g], on[:])
        cpool_cm.__exit__(None, None, None)
    nc.compile()
    return nc


def _prep_inputs(inputs):
    f = np.float32
    x = np.asarray(inputs["x"], f)
    wq, bq = np.asarray(inputs["wq"], f), np.asarray(inputs["bq"], f)
    wk, bk = np.asarray(inputs["wk"], f), np.asarray(inputs["bk"], f)
    wv, bv = np.asarray(inputs["wv"], f), np.asarray(inputs["bv"], f)
    wp, bp = np.asarray(inputs["wp"], f), np.asarray(inputs["bp"], f)
    ln1_g, ln1_b = np.asarray(inputs["ln1_g"], f), np.asarray(inputs["ln1_b"], f)
    ln2_g, ln2_b = np.asarray(inputs["ln2_g"], f), np.asarray(inputs["ln2_b"], f)
    wr, br = np.asarray(inputs["wr"], f), np.asarray(inputs["br"], f)
    w1, b1 = np.asarray(inputs["w1"], f), np.asarray(inputs["b1"], f)
    w2, b2 = np.asarray(inputs["w2"], f), np.asarray(inputs["b2"], f)

    def etile(a):  # [E, M] -> [128, 8*M]
        M = a.shape[1]
        return np.ascontiguousarray(
            a.reshape(8, 128, M).transpose(1, 0, 2).reshape(128, 8 * M))

    wq_f = wq.transpose(1, 0, 2).reshape(E, E)   # [e, h*64+d]
    wk_f = wk.transpose(1, 0, 2).reshape(E, E)
    wv_f = wv.transpose(1, 0, 2).reshape(E, E)
    wqkv = np.concatenate([wq_f, wk_f, wv_f], axis=1)        # [E, 3E]
    wqkv_t = etile(wqkv * SW_QKV).astype(FP8)                # [128, 8*3072]
    bqk = np.concatenate([bq.reshape(-1).reshape(8, 128).T,
                          bk.reshape(-1).reshape(8, 128).T],
                         axis=1).astype(f) * SW_QKV
    wp_t = etile(wp * SW_P).astype(FP8)                      # [128, 8*1024]
    w1e = (ln1_g[:, None] * w1).astype(f)                    # [n,E,FF]
    b1e = b1 + ln1_b @ w1                                    # [n,FF]
    wre = (ln1_g[:, None] * wr).astype(f)                    # [E,8]
    bre = br + ln1_b @ wr                                    # [8]
    ident = np.eye(128, dtype=BF16)

    in_maps = []
    for r in range(NCORE):
        b, p = r // 2, r % 2
        # interleaved token assignment: local s_loc <-> orig row 2*s_loc + p
        xs = np.ascontiguousarray(x[b, p::2, :])             # [512, E]
        xpart = np.ascontiguousarray(x[b, 1 - p::2, :])      # partner tokens
        xT_t = np.concatenate(
            [etile(np.ascontiguousarray(xs.T)),
             etile(np.ascontiguousarray(xpart.T))], axis=1).astype(FP8)
        xr_t = np.ascontiguousarray(
            xs.reshape(4, 128, 1024).transpose(1, 0, 2).reshape(128, 4096),
            f) * np.float32(SW_P)
        xr_t = xr_t.astype(BF16)
        # final-phase x rows: vt rows [128r,128r+128) of every batch
        sv = 128 * r + np.arange(128)
        orig_s = 2 * (sv % 512) + sv // 512
        xr2_t = np.ascontiguousarray(
            x[:, orig_s, :].transpose(1, 0, 2).reshape(128, 4096), f)
        # diagonal causal masks: half0 = own parity keys, half1 = partner
        masks = np.zeros((4, 128, 256), BF16)
        ti = np.arange(128)
        sj = np.arange(128)
        own = (ti[:, None] <= sj[None, :])
        part = (ti[:, None] <= sj[None, :]) if p == 1 else \
               (ti[:, None] < sj[None, :])
        for ss in range(4):
            masks[ss][:, 0:128] = own.astype(BF16)
            masks[ss][:, 128:256] = part.astype(BF16)
        perm = [r] + [i for i in range(NEXP) if i != r]
        wr_p = etile(wre[:, perm] * SW_R).astype(FP8)        # [128, 8*8]
        br_p = (bre[perm] * SW_R).reshape(1, 8).astype(BF16)
        w1_t = etile(w1e[r] * SW_1).astype(FP8)              # [128, 8*4096]
        b1_t = np.ascontiguousarray(
            b1e[r].reshape(32, 128).T, f) * np.float32(SW_1)  # [128, 32]
        w2_t = np.ascontiguousarray(
            w2[r].reshape(32, 128, 1024).transpose(1, 0, 2)
            .reshape(128, 32 * 1024) * SW_2).astype(FP8)
        in_maps.append({
            "xT": xT_t, "xr": xr_t, "xr2": xr2_t, "wqkv": wqkv_t, "bqk": bqk,
            "bv": (bv * SW_QKV).reshape(1, E).astype(BF16),
            "wp": wp_t, "bp": (bp * SW_P).reshape(1, E).astype(BF16),
            "masks": masks, "ident": ident,
            "w1": w1_t, "b1": b1_t, "w2": w2_t,
            "b2": (b2[r] * SW_1 * SW_2).reshape(1, E).astype(BF16),
            "wr": wr_p, "br": br_p,
            "g2": np.broadcast_to(ln2_g, (128, E)).astype(f).copy(),
            "bl2": np.broadcast_to(ln2_b, (128, E)).astype(f).copy(),
        })
    return in_maps


def kernel(**inputs):
    from concourse import bass_utils
    if "nc" not in _CACHE:
        _CACHE["nc"] = _build_program()
    nc = _CACHE["nc"]
    in_maps = _prep_inputs(inputs)
    res = bass_utils.run_bass_kernel_spmd(
        nc, in_maps, core_ids=list(range(NCORE)))
    # core r returns vt rows [128r, 128r+128) of every batch (interleaved map)
    full = np.empty((B, S, E), np.float32)
    for r in range(NCORE):
        o = res.results[r]["out"]                            # [4, 128, 1024]
        sv = 128 * r + np.arange(128)
        orig_s = 2 * (sv % 512) + sv // 512
        full[:, orig_s, :] = o
    return full


# revision 15
# speedup vs baseline: 1.0422x; 1.0422x over previous
"""Trainium2 Bass kernel for nn_Block_56616258896419 (moe_routing).

Self-contained: takes FULL inputs (as from setup_inputs()), returns FULL
[4,1024,1024] f32 output. Internally shards across 8 NeuronCores:
  - tokens 8-way (core r: batch r//2, sequence half r%2) for attention/LN
  - experts 8-way (core r computes expert r over ALL tokens) for the MoE
Heavy matmuls (QKV, proj, both expert GEMMs) run in fp8-e4m3 with
DoubleRow perf mode (2 MACs/cell/cycle). Scales fold into weight staging
(wqkv/wp/wr x64, w1 x16, w2 x32) and are undone inside existing ops:
LN1 is scale-invariant (residual x staged x64), the expert combine
multiplies by probs/512, q is scaled by 0.125/64^2 before softmax.
Collectives: 8-way AllGather of LN1'd activations (transposed, fp8),
4x 8-way ReduceScatter (bf16) of the prob-weighted expert outputs,
overlapped with MoE compute.
"""
import numpy as np
import ml_dtypes

B, S, E, H, HD, NEXP, FF = 4, 1024, 1024, 16, 64, 8, 4096
NCORE = 8
EPS = 1e-5
BF16 = ml_dtypes.bfloat16
FP8 = ml_dtypes.float8_e4m3

SW_QKV = 64.0
SW_P = 64.0
SW_R = 64.0
SW_1 = 16.0
SW_2 = 32.0
PCOL_DESCALE = 1.0 / (SW_1 * SW_2)
Q_SCALE = 0.125 / (SW_QKV * SW_QKV)
V_DESCALE = 1.0 / SW_QKV
R_DESCALE = 1.0 / SW_R

_CACHE = {}


def _build_program():
    import concourse.bacc as bacc
    import concourse.mybir as mybir
    import concourse.tile as tile

    dt = mybir.dt
    f32, bf, f8 = dt.float32, dt.bfloat16, dt.float8e4
    AF = mybir.ActivationFunctionType
    ALU = mybir.AluOpType
    DR = mybir.MatmulPerfMode.DoubleRow

    nc = bacc.Bacc("TRN2", target_bir_lowering=False, debug=False,
                   num_devices=NCORE)

    # ---------------- I/O ----------------
    def inp(name, shape, d):
        return nc.dram_tensor(name, shape, d, kind="ExternalInput").ap()

    xT_d = inp("xT", [128, 2 * 4096], f8)          # x^T [own|partner], e-tiled
    xr_d = inp("xr", [128, 4 * 1024], bf)         # 64*x token-major, tt blocks
    xr2_d = inp("xr2", [128, 4 * 1024], f32)       # x rows [b, 128r:128r+128]
    wqkv_d = inp("wqkv", [128, 8 * 3072], f8)      # 64*[E,3E] e-tiled
    bqk_d = inp("bqk", [128, 16], f32)             # 64*biases
    bv_d = inp("bv", [1, 1024], bf)                # 64*bv
    wp_d = inp("wp", [128, 8 * 1024], f8)          # 64*wp e-tiled
    bp_d = inp("bp", [1, 1024], bf)                # 64*bp
    masks_d = inp("masks", [4, 128, 256], bf)      # diag-pair 0/1 masks
    ident_d = inp("ident", [128, 128], bf)
    w1_d = inp("w1", [128, 8 * 4096], f8)          # 16*(ln1-folded), e-tiled
    b1_d = inp("b1", [128, 32], f32)               # 16*b1
    w2_d = inp("w2", [128, 32 * 1024], f8)         # 32*w2, ff-tiled
    b2_d = inp("b2", [1, 1024], bf)                # 512*b2
    wr_d = inp("wr", [128, 8 * 8], f8)             # 64*(ln1-folded), permuted
    br_d = inp("br", [1, 8], bf)                   # 64*br
    g2_d = inp("g2", [128, 1024], f32)             # ln2_g replicated
    bl2_d = inp("bl2", [128, 1024], f32)           # ln2_b replicated
    out_d = nc.dram_tensor("out", [4, 128, 1024], f32,
                           kind="ExternalOutput").ap()
    DBG = False
    if DBG:
        dbg_q = nc.dram_tensor("dbg_q", [128, 4096], bf,
                               kind="ExternalOutput").ap()
        dbg_k = nc.dram_tensor("dbg_k", [128, 8192], bf,
                               kind="ExternalOutput").ap()
        dbg_c = nc.dram_tensor("dbg_c", [128, 4096], f8,
                               kind="ExternalOutput").ap()
        dbg_h = nc.dram_tensor("dbg_h", [128, 4096], bf,
                               kind="ExternalOutput").ap()

    # ---------------- internal DRAM ----------------
    hag_inA = nc.dram_tensor("hag_inA", [8, 128, 256], f8).ap()
    hag_inB = nc.dram_tensor("hag_inB", [8, 128, 256], f8).ap()
    hag_outA = nc.dram_tensor("hag_outA", [8, 8, 128, 256], f8,
                              addr_space="Shared").ap()
    hag_outB = nc.dram_tensor("hag_outB", [8, 8, 128, 256], f8,
                              addr_space="Shared").ap()
    rs_ins = [nc.dram_tensor(f"rs_in{g}", [1024, 1024], bf).ap()
              for g in range(4)]
    rs_outs = [nc.dram_tensor(f"rs_out{g}", [128, 1024], bf).ap()
               for g in range(4)]

    with tile.TileContext(nc) as tc:
        cpool_cm = tc.tile_pool(name="cpool", bufs=1, side="left")
        cpool = cpool_cm.__enter__()
        ones_row = cpool.tile([1, 128], bf)
        nc.vector.memset(ones_row[:], 1.0)
        bqk_sb = cpool.tile([128, 16], f32)
        nc.sync.dma_start(bqk_sb[:], bqk_d[:])
        bv_sb = cpool.tile([1, 1024], bf)
        nc.sync.dma_start(bv_sb[:], bv_d[:])
        bp_sb = cpool.tile([1, 1024], bf)
        nc.sync.dma_start(bp_sb[:], bp_d[:])
        ident_sb = cpool.tile([128, 128], bf)
        nc.sync.dma_start(ident_sb[:], ident_d[:])
        wr_sb = cpool.tile([128, 64], f8)
        nc.sync.dma_start(wr_sb[:], wr_d[:])
        br_sb = cpool.tile([1, 8], bf)
        nc.sync.dma_start(br_sb[:], br_d[:])
        b1_sb = cpool.tile([128, 32], f32)
        nc.sync.dma_start(b1_sb[:], b1_d[:])
        b2_sb = cpool.tile([1, 1024], bf)
        nc.sync.dma_start(b2_sb[:], b2_d[:])

        # ===== phase 1+2: fused QKV + attention (q-block ss outer) =====
        qkv_cm = tc.tile_pool(name="qkv", bufs=1, side="right")
        qkv = qkv_cm.__enter__()
        xT_sb = qkv.tile([128, 8192], f8)
        nc.sync.dma_start(xT_sb[:], xT_d[:])
        wqkv_sb = qkv.tile([128, 24576], f8)
        for sect in (1, 2, 0):            # k first, then v, then q
            nc.sync.dma_start(
                wqkv_sb[:].rearrange("p (e s c) -> p e s c", e=8, s=3)[:, :, sect],
                wqkv_d[:].rearrange("p (e s c) -> p e s c", e=8, s=3)[:, :, sect])

        attn_cm = tc.tile_pool(name="attn", bufs=1, side="right")
        attn = attn_cm.__enter__()
        qT_sb = attn.tile([128, 4096], bf)
        kT_full = attn.tile([128, 8192], bf)     # [j][half*512 + s], 64-scaled
        # augmented V: per t-tile, 16 heads x (64 v-cols + 1 ones-col)
        v_aug = attn.tile([128, 8 * 1040], bf)
        for u in range(8):
            nc.vector.memset(
                v_aug[:, u * 1040: u * 1040 + 1040]
                .rearrange("p (h d) -> p h d", d=65)[:, :, 64:65], 1.0)
        mask_sb = attn.tile([128, 1024], bf)
        for ss in range(4):
            nc.sync.dma_start(mask_sb[:, ss * 256: ss * 256 + 256],
                              masks_d[ss])

        proj_cm = tc.tile_pool(name="proj", bufs=1, side="right")
        projp = proj_cm.__enter__()
        cat8_sb = projp.tile([128, 4096], f8)    # normalized heads^T (fp8)
        wp_sb = projp.tile([128, 8192], f8)
        nc.sync.dma_start(wp_sb[:], wp_d[:])
        x_sb = projp.tile([128, 4096], bf)       # 64*x
        nc.sync.dma_start(x_sb[:], xr_d[:])
        h_sb = projp.tile([128, 4096], bf)
        hT_stage = projp.tile([128, 4096], f8)

        # MoE weights prefetch - overlaps the whole attention phase
        moe_cm = tc.tile_pool(name="moe", bufs=1, side="left")
        moe = moe_cm.__enter__()
        w1_sb = moe.tile([128, 32768], f8)
        w2_sb = moe.tile([128, 32768], f8)
        for et in range(8):
            nc.sync.dma_start(w1_sb[:, et * 4096: et * 4096 + 4096],
                              w1_d[:, et * 4096: et * 4096 + 4096])
        for ft8 in range(8):
            nc.sync.dma_start(w2_sb[:, ft8 * 4096: ft8 * 4096 + 4096],
                              w2_d[:, ft8 * 4096: ft8 * 4096 + 4096])

        hTcp_cm = tc.tile_pool(name="hTcp", bufs=2, side="left")
        hTcp = hTcp_cm.__enter__()

        xT4 = xT_sb[:].rearrange("p (hf et t) -> p hf et t", hf=2, et=8)
        wq4 = wqkv_sb[:].rearrange("p (et s c) -> p et s c", et=8, s=3)
        cat4 = cat8_sb[:].rearrange("p (s2 jj t) -> p s2 jj t", s2=4, jj=8)
        wp4 = wp_sb[:].rearrange("p (et c) -> p et c", et=8)
        hT3 = hT_stage[:].rearrange("p (et t) -> p et t", et=8)

        # block order within a subtile: non-diag (ph*ss+tl), diag at 2ss+ph
        def blk(ss, ph, tl):
            return 2 * ss + ph if tl == ss else ph * ss + tl

        # v c1 emission order inside ss=0 (u needed by av(ss,.): u in
        # {tl, 4+tl : tl<=ss}; c1 heads 8..15 needed from j=4 of each ss)
        V1_ORDER = [0, 4, 1, 5, 2, 6, 3, 7]

        def emit_v(half, tt, c, scp2, ps_sc2):
            v_ps = ps_sc2.tile([128, 1024], f32, tag="sc",
                               name=f"v{half}_{tt}_{c}")[:, 0:512]
            for g in range(4):
                nc.tensor.matmul(
                    v_ps[:],
                    xT4[:, half, 2 * g:2 * g + 2, tt * 128: tt * 128 + 128],
                    wq4[:, 2 * g:2 * g + 2, 2, c * 512: c * 512 + 512],
                    start=(g == 0), stop=False, perf_mode=DR)
            nc.tensor.matmul(
                v_ps[:], ones_row[:, 0:128],
                bv_sb[:, c * 512: c * 512 + 512],
                start=False, stop=True)
            u = half * 4 + tt
            v_st = scp2.tile([128, 512], bf, tag="v_st", bufs=2,
                             name=f"vs{half}_{tt}_{c}")
            nc.vector.tensor_scalar_mul(v_st[:], v_ps[:], V_DESCALE)
            nc.vector.tensor_copy(
                v_aug[:, u * 1040: u * 1040 + 1040]
                .rearrange("p (h d) -> p h d", d=65)[:, 8 * c: 8 * c + 8, 0:64],
                v_st[:].rearrange("p (h d) -> p h d", d=64))

        with tc.tile_pool(name="sc", bufs=4, side="left") as scp, \
             tc.tile_pool(name="ps_sc", bufs=2, space="PSUM") as ps_sc, \
             tc.tile_pool(name="ps_av", bufs=2, space="PSUM") as ps_av, \
             tc.tile_pool(name="ps_pr", bufs=1, space="PSUM") as ps_pr:
            # v c0 for all 8 u-tiles up front (needed by av(0, j<4))
            for half in range(2):
                for tt in range(4):
                    emit_v(half, tt, 0, scp, ps_sc)

            av_tiles = {}
            exp_tiles = {}
            preloaded = {}

            def preload_chunk(ci, q, hf):
                t = hTcp.tile([128, 4096], f8, tag="hT_c", name=f"hTc{ci}")
                hsrc = hag_outA if hf == 0 else hag_outB
                for et in range(8):
                    for si in range(2):
                        nc.sync.dma_start(
                            t[:, et * 512 + si * 256:
                              et * 512 + si * 256 + 256],
                            hsrc[2 * q + si, et])
                preloaded[ci] = t

            def emit_sc_exp(ss, j):
                scs = []
                for h01 in range(2):
                    scs.append(ps_sc.tile([128, 1024], f32, tag="sc",
                                          name=f"sc{ss}_{j}_{h01}"))
                for ph in range(2):
                    for tl in range(ss + 1):
                        m = blk(ss, ph, tl)
                        for h01 in range(2):
                            po = 64 * h01
                            nc.tensor.matmul(
                                scs[h01][:, m * 128: m * 128 + 128],
                                kT_full[po:po + 64,
                                        j * 1024 + ph * 512 + tl * 128:
                                        j * 1024 + ph * 512 + tl * 128 + 128],
                                qT_sb[po:po + 64, j * 512 + ss * 128:
                                      j * 512 + ss * 128 + 128],
                                start=True, stop=True)
                exps = []
                for h01 in range(2):
                    expT = scp.tile([128, 1024], bf, tag="expT", bufs=3,
                                    name=f"ex{ss}_{j}_{h01}")
                    exps.append(expT)
                    nc.scalar.activation(
                        expT[:, 0: (2 * ss + 2) * 128],
                        scs[h01][:, 0: (2 * ss + 2) * 128], AF.Exp)
                    nc.vector.tensor_tensor(
                        expT[:, 2 * ss * 128: 2 * ss * 128 + 256],
                        expT[:, 2 * ss * 128: 2 * ss * 128 + 256],
                        mask_sb[:, ss * 256: ss * 256 + 256], op=ALU.mult)
                exp_tiles[(j % 4, 0)] = exps[0]
                exp_tiles[(j % 4, 1)] = exps[1]

            def emit_av(ss, j):
                if j % 4 == 0:
                    av_tiles[0] = ps_av.tile([65, 512], f32, tag="av",
                                             name=f"av{ss}_{j // 4}_0")
                    av_tiles[1] = ps_av.tile([65, 512], f32, tag="av",
                                             name=f"av{ss}_{j // 4}_1")
                exps = [exp_tiles[(j % 4, 0)], exp_tiles[(j % 4, 1)]]
                nblk = 2 * (ss + 1)
                bi = 0
                for ph in range(2):
                    for tl in range(ss + 1):
                        m = blk(ss, ph, tl)
                        u = ph * 4 + tl
                        bi += 1
                        for h01 in range(2):
                            h = 2 * j + h01
                            nc.tensor.matmul(
                                av_tiles[h01][:, (j % 4) * 128:
                                              (j % 4) * 128 + 128],
                                v_aug[:, u * 1040 + h * 65:
                                      u * 1040 + h * 65 + 65],
                                exps[h01][:, m * 128: m * 128 + 128],
                                start=(bi == 1), stop=(bi == nblk))

            def emit_cat(ss, jh):
                # normalize 4 head-pairs' q-slice from av PSUM into cat8.
                # cat8 layout is ss-major: col = ss*1024 + j*128 + t, so this
                # drain is a contiguous [128, 512] block (2D ops only; ACT
                # stages the PSUM reads into SBUF as in v2).
                bc = ps_sc.tile([128, 1024], f32, tag="sc",
                                name=f"bc{ss}_{jh}")[:, 0:512]
                catT_st = scp.tile([128, 512], bf, tag="catT", bufs=2,
                                   name=f"cst{ss}_{jh}")
                for h01 in range(2):
                    nc.scalar.copy(catT_st[64 * h01: 64 * h01 + 64, :],
                                   av_tiles[h01][0:64, :])
                    sm_f = scp.tile([1, 512], f32, tag="sm_f", bufs=1,
                                    name=f"smf{ss}_{jh}_{h01}")
                    nc.scalar.copy(sm_f[:], av_tiles[h01][64:65, :])
                    rc_f = scp.tile([1, 512], f32, tag="rc_f", bufs=1,
                                    name=f"rcf{ss}_{jh}_{h01}")
                    nc.vector.reciprocal_approx_fast(rc_f[:], sm_f[:])
                    rc_h = scp.tile([1, 512], bf, tag="rc_h", bufs=1,
                                    name=f"rc{ss}_{jh}_{h01}")
                    nc.vector.tensor_copy(rc_h[:], rc_f[:])
                    nc.tensor.matmul(bc[64 * h01: 64 * h01 + 64, :],
                                     ones_row[:, 0:64], rc_h[:],
                                     start=True, stop=True)
                nc.vector.tensor_tensor(
                    cat8_sb[:, ss * 1024 + jh * 512:
                            ss * 1024 + jh * 512 + 512],
                    catT_st[:], bc[:], op=ALU.mult)

            def emit_proj_ln(ss):
                tt = ss
                y_sb = prp.tile([128, 1024], f32, tag="y", bufs=1)
                for ec in range(2):
                    ao_ps = ps_pr.tile([128, 512], f32, tag="ao")
                    for g in range(4):
                        nc.tensor.matmul(
                            ao_ps[:],
                            cat4[:, tt, 2 * g:2 * g + 2, :],
                            wp4[:, 2 * g:2 * g + 2, ec * 512: ec * 512 + 512],
                            start=(g == 0), stop=False, perf_mode=DR)
                    nc.tensor.matmul(
                        ao_ps[:], ones_row[:, 0:128],
                        bp_sb[:, ec * 512: ec * 512 + 512],
                        start=False, stop=True)
                    nc.vector.tensor_tensor(
                        y_sb[:, ec * 512: ec * 512 + 512], ao_ps[:],
                        x_sb[:, tt * 1024 + ec * 512: tt * 1024 + ec * 512 + 512],
                        op=ALU.add)
                # LN1 stats (y is 64*(attn_out+x); LN is scale-invariant)
                mean = prp.tile([128, 1], f32, tag="mean")
                nc.vector.reduce_sum(mean[:], y_sb[:], axis=mybir.AxisListType.X)
                nc.vector.tensor_scalar_mul(mean[:], mean[:], 1.0 / 1024.0)
                sq = scp.tile([128, 1024], bf, tag="expT", bufs=3,
                              name=f"sq{ss}")
                sqs = prp.tile([128, 1], f32, tag="sqs")
                nc.scalar.activation(sq[:], y_sb[:], AF.Square,
                                     accum_out=sqs[:])
                m2 = prp.tile([128, 1], f32, tag="m2")
                nc.scalar.activation(m2[:], mean[:], AF.Square)
                var = prp.tile([128, 1], f32, tag="var")
                nc.vector.tensor_scalar(var[:], sqs[:], 1.0 / 1024.0, EPS,
                                        op0=ALU.mult, op1=ALU.add)
                nc.vector.tensor_tensor(var[:], var[:], m2[:], op=ALU.subtract)
                std = prp.tile([128, 1], f32, tag="std")
                nc.scalar.activation(std[:], var[:], AF.Sqrt)
                rstd = prp.tile([128, 1], f32, tag="rstd")
                nc.vector.reciprocal(rstd[:], std[:])
                nc.vector.tensor_scalar(
                    h_sb[:, tt * 1024: tt * 1024 + 1024], y_sb[:],
                    mean[:], rstd[:], op0=ALU.subtract, op1=ALU.mult)
                # transpose h tile -> hT (fp8 via the copy)
                tp = ps_pr.tile([128, 1024], bf, tag="tp")
                for et in range(8):
                    nc.tensor.transpose(
                        tp[:, et * 128: et * 128 + 128],
                        h_sb[:, tt * 1024 + et * 128: tt * 1024 + et * 128 + 128],
                        ident_sb[:])
                nc.scalar.copy(
                    hT3[:, :, tt * 128: tt * 128 + 128],
                    tp[:].rearrange("p (et t) -> p et t", et=8))
                if tt == 1:
                    for et in range(8):
                        nc.gpsimd.dma_start(
                            hag_inA[et],
                            hT_stage[:, et * 512: et * 512 + 256])
                    nc.gpsimd.collective_compute(
                        "AllGather", mybir.AluOpType.bypass,
                        replica_groups=[list(range(8))],
                        ins=[hag_inA.opt()], outs=[hag_outA.opt()])
                    preload_chunk(0, 0, 0)
                    preload_chunk(1, 1, 0)
                if tt == 3:
                    for et in range(8):
                        nc.gpsimd.dma_start(
                            hag_inB[et],
                            hT_stage[:, et * 512 + 256: et * 512 + 512])
                    nc.gpsimd.collective_compute(
                        "AllGather", mybir.AluOpType.bypass,
                        replica_groups=[list(range(8))],
                        ins=[hag_inB.opt()], outs=[hag_outB.opt()])

            prp_cm = tc.tile_pool(name="prw", bufs=2, side="left")
            prp = prp_cm.__enter__()
            for ss in range(4):
                for j in range(8):
                    if ss == 0:
                        # interleave K (both halves) / Q / V-c1 for this j
                        k_dsts = [
                            (1, kT_full[:, j * 1024: j * 1024 + 512], 8 + j),
                            (1, kT_full[:, j * 1024 + 512: j * 1024 + 1024],
                             8 + j),
                            (0, qT_sb[:, j * 512: j * 512 + 512], j)]
                        for li, (sect, dst, bcol) in enumerate(k_dsts):
                            half = li if li < 2 else 0
                            kq_ps = ps_sc.tile([128, 1024], f32, tag="sc",
                                               name=f"kq{j}_{li}")[:, 0:512]
                            for g in range(4):
                                nc.tensor.matmul(
                                    kq_ps[:],
                                    wq4[:, 2 * g:2 * g + 2, sect,
                                        j * 128: j * 128 + 128],
                                    xT4[:, half, 2 * g:2 * g + 2, :],
                                    start=(g == 0), stop=(g == 3),
                                    perf_mode=DR)
                            if sect == 1:
                                nc.vector.tensor_scalar(
                                    dst, kq_ps[:],
                                    bqk_sb[:, bcol: bcol + 1], None,
                                    op0=ALU.add)
                            else:
                                nc.vector.tensor_scalar(
                                    dst, kq_ps[:],
                                    bqk_sb[:, bcol: bcol + 1], Q_SCALE,
                                    op0=ALU.add, op1=ALU.mult)
                        u = V1_ORDER[j]
                        emit_v(u // 4, u % 4, 1, scp, ps_sc)
                    emit_sc_exp(ss, j)
                    if j >= 1:
                        emit_av(ss, j - 1)
                    if j == 3 + 1:
                        emit_cat(ss, 0)
                emit_av(ss, 7)
                emit_cat(ss, 1)
                emit_proj_ln(ss)
            prp_cm.__exit__(None, None, None)
            if DBG:
                nc.sync.dma_start(dbg_q[:], qT_sb[:])
                nc.sync.dma_start(dbg_k[:], kT_full[:])
                nc.sync.dma_start(dbg_c[:], cat8_sb[:])
                nc.sync.dma_start(dbg_h[:], h_sb[:])  # noqa
        proj_cm.__exit__(None, None, None)
        attn_cm.__exit__(None, None, None)
        qkv_cm.__exit__(None, None, None)

        # ===== phase 3: MoE (expert r over all tokens, 512-token chunks) ====
        # chunk (q, hf) = tokens of cores 2q,2q+1, local-token half hf
        w1v = w1_sb[:].rearrange("p (et f) -> p et f", et=8)
        w2v = w2_sb[:].rearrange("p (ft e) -> p ft e", ft=32)
        with tc.tile_pool(name="mchunk", bufs=2, side="left") as mck, \
             tc.tile_pool(name="ps_md", bufs=2, space="PSUM") as ps_md, \
             tc.tile_pool(name="ps_eo", bufs=2, space="PSUM") as ps_eo:
            chunks = [(0, 0), (1, 0), (0, 1), (1, 1),
                      (2, 0), (3, 0), (2, 1), (3, 1)]
            rs_fire = {2: 0, 3: 1, 6: 2, 7: 3}
            for ci, (q, hf) in enumerate(chunks):
                if ci in preloaded:
                    hT_c = preloaded.pop(ci)
                else:
                    hT_c = hTcp.tile([128, 4096], f8, tag="hT_c",
                                     name=f"hTc{ci}")
                    hsrc = hag_outA if hf == 0 else hag_outB
                    for et in range(8):
                        for si in range(2):
                            nc.gpsimd.dma_start(
                                hT_c[:, et * 512 + si * 256:
                                     et * 512 + si * 256 + 256],
                                hsrc[2 * q + si, et])
                h8v = hT_c[:].rearrange("p (et t) -> p et t", et=8)
                pcol = mck.tile([128, 4], f32, tag="pcol")
                for th in range(4):
                    lg_ps = ps_eo.tile([128, 8], f32, tag="lg")
                    for et in range(8):
                        nc.tensor.matmul(
                            lg_ps[:],
                            hT_c[:, et * 512 + th * 128: et * 512 + th * 128 + 128],
                            wr_sb[:, et * 8: et * 8 + 8],
                            start=(et == 0), stop=False)
                    nc.tensor.matmul(lg_ps[:], ones_row[:, 0:128], br_sb[:],
                                     start=False, stop=True)
                    pe = mck.tile([128, 8], f32, tag="pe")
                    ps = mck.tile([128, 1], f32, tag="ps")
                    nc.scalar.activation(pe[:], lg_ps[:], AF.Exp,
                                         scale=R_DESCALE, accum_out=ps[:])
                    pr = mck.tile([128, 1], f32, tag="pr")
                    nc.vector.reciprocal(pr[:], ps[:])
                    nc.vector.tensor_scalar(pcol[:, th:th + 1], pe[:, 0:1],
                                            pr[:], PCOL_DESCALE,
                                            op0=ALU.mult, op1=ALU.mult)
                midT8 = mck.tile([128, 16384], f8, tag="midT", bufs=2)
                for ft in range(32):
                    md_ps = ps_md.tile([128, 512], f32, tag="md")
                    for g in range(4):
                        nc.tensor.matmul(
                            md_ps[:],
                            w1v[:, 2 * g:2 * g + 2, ft * 128: ft * 128 + 128],
                            h8v[:, 2 * g:2 * g + 2, :],
                            start=(g == 0), stop=(g == 3), perf_mode=DR)
                    if ft % 2 == 0:
                        nc.scalar.activation(
                            midT8[:, ft * 512: ft * 512 + 512], md_ps[:],
                            AF.Relu, bias=b1_sb[:, ft: ft + 1])
                    else:
                        nc.vector.tensor_scalar(
                            midT8[:, ft * 512: ft * 512 + 512], md_ps[:],
                            b1_sb[:, ft: ft + 1], 0.0,
                            op0=ALU.add, op1=ALU.max)
                m8v = midT8[:].rearrange("p (ft t) -> p ft t", ft=32)
                eo_sb = mck.tile([128, 4096], bf, tag="eo", bufs=2)
                for th in range(4):
                    for ec in range(2):
                        eo_ps = ps_eo.tile([128, 512], f32, tag="eo_ps")
                        for g in range(16):
                            nc.tensor.matmul(
                                eo_ps[:],
                                m8v[:, 2 * g:2 * g + 2, th * 128: th * 128 + 128],
                                w2v[:, 2 * g:2 * g + 2, ec * 512: ec * 512 + 512],
                                start=(g == 0), stop=False, perf_mode=DR)
                        nc.tensor.matmul(
                            eo_ps[:], ones_row[:, 0:128],
                            b2_sb[:, ec * 512: ec * 512 + 512],
                            start=False, stop=True)
                        if ec == 0:
                            nc.scalar.activation(
                                eo_sb[:, th * 1024 + ec * 512:
                                      th * 1024 + ec * 512 + 512],
                                eo_ps[:], AF.Identity,
                                scale=pcol[:, th: th + 1])
                        else:
                            nc.vector.tensor_scalar_mul(
                                eo_sb[:, th * 1024 + ec * 512:
                                      th * 1024 + ec * 512 + 512],
                                eo_ps[:], pcol[:, th: th + 1])
                    r0 = 512 * (th // 2) + hf * 256 + 128 * (th % 2)
                    nc.sync.dma_start(
                        rs_ins[q][r0: r0 + 128, :],
                        eo_sb[:, th * 1024: th * 1024 + 1024])
                if ci in rs_fire:
                    g = rs_fire[ci]
                    nc.gpsimd.collective_compute(
                        "ReduceScatter", mybir.AluOpType.add,
                        replica_groups=[list(range(8))],
                        ins=[rs_ins[g].opt()], outs=[rs_outs[g].opt()])
        hTcp_cm.__exit__(None, None, None)
        moe_cm.__exit__(None, None, None)

        # ============ phase 4: residual + LN2 (per RS group/batch) ============
        with tc.tile_pool(name="fin", bufs=2, side="left") as fin:
            x2_sb = fin.tile([128, 4096], f32, bufs=1)
            nc.sync.dma_start(x2_sb[:], xr2_d[:])
            g2_sb = fin.tile([128, 1024], f32, bufs=1)
            nc.sync.dma_start(g2_sb[:], g2_d[:])
            bl2_sb = fin.tile([128, 1024], f32, bufs=1)
            nc.sync.dma_start(bl2_sb[:], bl2_d[:])
            for g in range(4):
                y2b = fin.tile([128, 1024], bf, tag="y2b")
                nc.sync.dma_start(y2b[:], rs_outs[g][:])
                y2 = fin.tile([128, 1024], f32, tag="y2")
                nc.vector.tensor_tensor(
                    y2[:], y2b[:], x2_sb[:, g * 1024: g * 1024 + 1024],
                    op=ALU.add)
                mean = fin.tile([128, 1], f32, tag="mean2")
                nc.vector.reduce_sum(mean[:], y2[:], axis=mybir.AxisListType.X)
                nc.vector.tensor_scalar_mul(mean[:], mean[:], 1.0 / 1024.0)
                sq = fin.tile([128, 1024], f32, tag="sq2")
                sqs = fin.tile([128, 1], f32, tag="sqs2")
                nc.scalar.activation(sq[:], y2[:], AF.Square, accum_out=sqs[:])
                m2 = fin.tile([128, 1], f32, tag="m22")
                nc.scalar.activation(m2[:], mean[:], AF.Square)
                var = fin.tile([128, 1], f32, tag="var2")
                nc.vector.tensor_scalar(var[:], sqs[:], 1.0 / 1024.0, EPS,
                                        op0=ALU.mult, op1=ALU.add)
                nc.vector.tensor_tensor(var[:], var[:], m2[:], op=ALU.subtract)
                std = fin.tile([128, 1], f32, tag="std2")
                nc.scalar.activation(std[:], var[:], AF.Sqrt)
                rstd = fin.tile([128, 1], f32, tag="rstd2")
                nc.vector.reciprocal(rstd[:], std[:])
                on = fin.tile([128, 1024], f32, tag="on")
                nc.vector.tensor_scalar(on[:], y2[:], mean[:], rstd[:],
                                        op0=ALU.subtract, op1=ALU.mult)
                nc.vector.tensor_tensor(on[:], on[:], g2_sb[:], op=ALU.mult)
                nc.vector.tensor_tensor(on[:], on[:], bl2_sb[:], op=ALU.add)
                nc.sync.dma_start(out_d[g], on[:])
        cpool_cm.__exit__(None, None, None)
    nc.compile()
    return nc


def _prep_inputs(inputs):
    f = np.float32
    x = np.asarray(inputs["x"], f)
    wq, bq = np.asarray(inputs["wq"], f), np.asarray(inputs["bq"], f)
    wk, bk = np.asarray(inputs["wk"], f), np.asarray(inputs["bk"], f)
    wv, bv = np.asarray(inputs["wv"], f), np.asarray(inputs["bv"], f)
    wp, bp = np.asarray(inputs["wp"], f), np.asarray(inputs["bp"], f)
    ln1_g, ln1_b = np.asarray(inputs["ln1_g"], f), np.asarray(inputs["ln1_b"], f)
    ln2_g, ln2_b = np.asarray(inputs["ln2_g"], f), np.asarray(inputs["ln2_b"], f)
    wr, br = np.asarray(inputs["wr"], f), np.asarray(inputs["br"], f)
    w1, b1 = np.asarray(inputs["w1"], f), np.asarray(inputs["b1"], f)
    w2, b2 = np.asarray(inputs["w2"], f), np.asarray(inputs["b2"], f)

    def etile(a):  # [E, M] -> [128, 8*M]
        M = a.shape[1]
        return np.ascontiguousarray(
            a.reshape(8, 128, M).transpose(1, 0, 2).reshape(128, 8 * M))

    wq_f = wq.transpose(1, 0, 2).reshape(E, E)   # [e, h*64+d]
    wk_f = wk.transpose(1, 0, 2).reshape(E, E)
    wv_f = wv.transpose(1, 0, 2).reshape(E, E)
    wqkv = np.concatenate([wq_f, wk_f, wv_f], axis=1)        # [E, 3E]
    wqkv_t = etile(wqkv * SW_QKV).astype(FP8)                # [128, 8*3072]
    bqk = np.concatenate([bq.reshape(-1).reshape(8, 128).T,
                          bk.reshape(-1).reshape(8, 128).T],
                         axis=1).astype(f) * SW_QKV
    wp_t = etile(wp * SW_P).astype(FP8)                      # [128, 8*1024]
    w1e = (ln1_g[:, None] * w1).astype(f)                    # [n,E,FF]
    b1e = b1 + ln1_b @ w1                                    # [n,FF]
    wre = (ln1_g[:, None] * wr).astype(f)                    # [E,8]
    bre = br + ln1_b @ wr                                    # [8]
    ident = np.eye(128, dtype=BF16)

    in_maps = []
    for r in range(NCORE):
        b, p = r // 2, r % 2
        # interleaved token assignment: local s_loc <-> orig row 2*s_loc + p
        xs = np.ascontiguousarray(x[b, p::2, :])             # [512, E]
        xpart = np.ascontiguousarray(x[b, 1 - p::2, :])      # partner tokens
        xT_t = np.concatenate(
            [etile(np.ascontiguousarray(xs.T)),
             etile(np.ascontiguousarray(xpart.T))], axis=1).astype(FP8)
        xr_t = np.ascontiguousarray(
            xs.reshape(4, 128, 1024).transpose(1, 0, 2).reshape(128, 4096),
            f) * np.float32(SW_P)
        xr_t = xr_t.astype(BF16)
        # final-phase x rows: vt rows [128r,128r+128) of every batch
        sv = 128 * r + np.arange(128)
        orig_s = 2 * (sv % 512) + sv // 512
        xr2_t = np.ascontiguousarray(
            x[:, orig_s, :].transpose(1, 0, 2).reshape(128, 4096), f)
        # diagonal causal masks: half0 = own parity keys, half1 = partner
        masks = np.zeros((4, 128, 256), BF16)
        ti = np.arange(128)
        sj = np.arange(128)
        own = (ti[:, None] <= sj[None, :])
        part = (ti[:, None] <= sj[None, :]) if p == 1 else \
               (ti[:, None] < sj[None, :])
        for ss in range(4):
            masks[ss][:, 0:128] = own.astype(BF16)
            masks[ss][:, 128:256] = part.astype(BF16)
        perm = [r] + [i for i in range(NEXP) if i != r]
        wr_p = etile(wre[:, perm] * SW_R).astype(FP8)        # [128, 8*8]
        br_p = (bre[perm] * SW_R).reshape(1, 8).astype(BF16)
        w1_t = etile(w1e[r] * SW_1).astype(FP8)              # [128, 8*4096]
        b1_t = np.ascontiguousarray(
            b1e[r].reshape(32, 128).T, f) * np.float32(SW_1)  # [128, 32]
        w2_t = np.ascontiguousarray(
            w2[r].reshape(32, 128, 1024).transpose(1, 0, 2)
            .reshape(128, 32 * 1024) * SW_2).astype(FP8)
        in_maps.append({
            "xT": xT_t, "xr": xr_t, "xr2": xr2_t, "wqkv": wqkv_t, "bqk": bqk,
            "bv": (bv * SW_QKV).reshape(1, E).astype(BF16),
            "wp": wp_t, "bp": (bp * SW_P).reshape(1, E).astype(BF16),
            "masks": masks, "ident": ident,
            "w1": w1_t, "b1": b1_t, "w2": w2_t,
            "b2": (b2[r] * SW_1 * SW_2).reshape(1, E).astype(BF16),
            "wr": wr_p, "br": br_p,
            "g2": np.broadcast_to(ln2_g, (128, E)).astype(f).copy(),
            "bl2": np.broadcast_to(ln2_b, (128, E)).astype(f).copy(),
        })
    return in_maps


def kernel(**inputs):
    from concourse import bass_utils
    if "nc" not in _CACHE:
        _CACHE["nc"] = _build_program()
    nc = _CACHE["nc"]
    in_maps = _prep_inputs(inputs)
    res = bass_utils.run_bass_kernel_spmd(
        nc, in_maps, core_ids=list(range(NCORE)))
    # core r returns vt rows [128r, 128r+128) of every batch (interleaved map)
    full = np.empty((B, S, E), np.float32)
    for r in range(NCORE):
        o = res.results[r]["out"]                            # [4, 128, 1024]
        sv = 128 * r + np.arange(128)
        orig_s = 2 * (sv % 512) + sv // 512
        full[:, orig_s, :] = o
    return full
